# revision 2
# baseline (speedup 1.0000x reference)
"""Trainium2 Bass kernel V2 for nn_AdjacencyConv (GNN message passing).

Reference computation:
    msg  = relu(concat[x[src], x_bridge[bri]] @ lin_w.T + lin_b)   # [E, D]
    agg  = segment_sum(msg, dst, N)                                # [N, D]
    out  = agg + (1+eps)*x
    h    = relu(BN(out @ w1.T + b1)); h = relu(BN(h @ w2.T + b2))  # train-mode BN

V2 device algorithm (8-core SPMD, edges sharded by dst node-tile):
  The per-edge linear factorizes msg = relu(xW[src] + eW[bri]) with
  xW = x @ Wx.T, eW = x_bridge @ Wb.T + b. The V1 kernel gathered BOTH
  rows per edge via Pool-engine dma_gather; Pool descriptor generation
  (~2.3 ns/row) was 85% busy and the bottleneck.

  V2 removes the x-side gather entirely:
    - Edge slots are sorted (dst-tile, src) and batched 128 at a time
      with src-span < 256 (a window PAIR of the xW table).
    - xW lives in SBUF [128, 80*128] bf16; each batch's x-rows are
      produced by 2 accumulated PE one-hot matmuls ("expand"):
      psX[slot,d] = sel_k[u,slot]^T @ xW[w0+k][u,d]. The window offset
      w0 differs per core under one SPMD program, so the matmul rhs AP
      uses a PE register loaded from a per-core offset table (woffs).
    - sel one-hots are host-built fp8 (exact 0/1) and streamed from
      DRAM; mixed fp8 lhsT x bf16 rhs matmul keeps full precision.
    - Only eW[bri] is gathered (1 descriptor/edge, halving Pool work).
  add+relu on DVE/Scalar, dst-scatter via one-hot matmul into PSUM
  aggT[d,v], residual + first MLP linear per finished tile, BN stats
  partials — all as V1. The BN tail stays as 2 follow-up NEFFs with
  host-stitched [128,2] stat reductions.
"""

import os
import numpy as np
import ml_dtypes

BF16 = ml_dtypes.bfloat16
FP8 = ml_dtypes.float8_e4m3fn
N, NB, E, D = 10000, 20000, 640000, 128
P = 128
NCORES = 8
NTC = 10                    # node tiles per core
N2 = NCORES * NTC * P       # 10240 padded x nodes -> 80 xW windows
NWX = N2 // P
NB2 = 20096                 # x_bridge padded to multiple of 128
BN_EPS = 1e-5
CHUNK = int(os.environ.get("KCHUNK", "16"))   # batches per gather/sel/oh group
GBUFS = int(os.environ.get("KGBUFS", "8"))
NQ = 4

_cache = {}
_cache2 = {}
last_results = None


def _pack_idx(idx16):
    """Wrap an int16 index vector for dma_gather: [16, n/16] replicated x8."""
    w = idx16.reshape(-1, 16).T
    return np.tile(w, (8, 1)).copy()


def _host_prep(inputs):
    x = np.asarray(inputs["x"], np.float32)
    xb = np.asarray(inputs["x_bridge"], np.float32)
    ei = np.asarray(inputs["edge_index"])
    bri = np.asarray(inputs["bridge_index"]).astype(np.int64)
    lin_w = np.asarray(inputs["lin_w"], np.float32)
    lin_b = np.asarray(inputs["lin_b"], np.float32)
    eps = float(np.asarray(inputs["eps"]).reshape(-1)[0])
    w1 = np.asarray(inputs["w1"], np.float32)
    g1 = np.asarray(inputs["g1"], np.float32)
    beta1 = np.asarray(inputs["beta1"], np.float32)
    w2 = np.asarray(inputs["w2"], np.float32)
    g2 = np.asarray(inputs["g2"], np.float32)
    beta2 = np.asarray(inputs["beta2"], np.float32)

    src = ei[0].astype(np.int64)
    dst = ei[1].astype(np.int64)

    # sort edges by dst, bucket into 128-node tiles; per tile sort by src
    order = np.argsort(dst, kind="stable")
    dsts = dst[order]
    srcs = src[order]
    bris = bri[order]
    gt_bounds = np.searchsorted(dsts, np.arange(NCORES * NTC + 1) * P)

    # batch formation per (core, tile): 128 slots, src-span < 256
    per_ct = [[[] for _ in range(NTC)] for _ in range(NCORES)]
    for gt in range(NCORES * NTC):
        c, lt = divmod(gt, NTC)
        a, b = int(gt_bounds[gt]), int(gt_bounds[gt + 1])
        s = srcs[a:b]
        e = bris[a:b]
        dl = (dsts[a:b] - gt * P).astype(np.float32)
        o2 = np.argsort(s, kind="stable")
        s, e, dl = s[o2], e[o2], dl[o2]
        n = len(s)
        i = 0
        while i < n:
            w0 = int(s[i]) // P
            hi = int(np.searchsorted(s, (w0 + 2) * P))
            j = min(i + P, hi)
            per_ct[c][lt].append((s[i:j], e[i:j], dl[i:j], w0))
            i = j

    NB_t = [max(len(per_ct[c][lt]) for c in range(NCORES)) for lt in range(NTC)]
    NB_t[-1] += (-sum(NB_t)) % CHUNK
    NBAT = sum(NB_t)
    S = NBAT * P

    srcrel = np.zeros((NCORES, S), np.int32)
    bri_pad = np.zeros((NCORES, S), np.int64)
    dloc_pad = np.full((NCORES, S), 999.0, np.float32)
    w0s = np.zeros((NCORES, NBAT), np.int32)
    for c in range(NCORES):
        B = 0
        for lt in range(NTC):
            for k in range(NB_t[lt]):
                off = B * P
                if k < len(per_ct[c][lt]):
                    s, e, dl, w0 = per_ct[c][lt][k]
                    m = len(s)
                    srcrel[c, off:off + m] = s - w0 * P
                    bri_pad[c, off:off + m] = e
                    dloc_pad[c, off:off + m] = dl
                    w0s[c, B] = w0
                B += 1
    # sel one-hots, fp8: sel_d[u, (B*2+k)*128+slot] = (srcrel[B*128+slot]==u+128k)
    sels = []
    uu = np.arange(2 * P, dtype=np.int32)[:, None]
    for c in range(NCORES):
        cmp = (uu == srcrel[c][None, :])
        sel = cmp.reshape(2, P, NBAT, P).transpose(1, 2, 0, 3).reshape(P, -1)
        sels.append(sel.astype(FP8))
    # dstloc transposed: [128, NBAT], column B = batch B's 128 local-dst values
    dloc_T = np.ascontiguousarray(
        dloc_pad.reshape(NCORES, NBAT, P).transpose(0, 2, 1)).astype(BF16)
    # window element offsets per (batch, pass): (w0+k)*128
    woffs = np.zeros((NCORES, 1, 2 * NBAT), np.int32)
    woffs[:, 0, 0::2] = w0s * P
    woffs[:, 0, 1::2] = (w0s + 1) * P

    # feature-major padded inputs for the table builds (bf16)
    xt = np.zeros((D, N2), BF16)
    xt[:, :N] = x.T.astype(BF16)
    xbt = np.zeros((D, NB2), BF16)
    xbt[:, :NB] = xb.T.astype(BF16)

    wxt = np.ascontiguousarray(lin_w[:, :D].T).astype(BF16)   # [in_f, out]
    wbt = np.ascontiguousarray(lin_w[:, D:].T).astype(BF16)
    w1t = np.ascontiguousarray(w1.T)
    w2t = np.ascontiguousarray(w2.T)
    linbb = np.tile(lin_b[None, :], (P, 1)).astype(np.float32)   # [128, 128]
    iotab = np.tile(np.arange(P, dtype=np.float32)[None, :],
                    (P, CHUNK)).astype(BF16)
    ident = np.eye(P, dtype=np.float32)

    g1c = np.ascontiguousarray(g1[:, None])
    b1c = np.ascontiguousarray(beta1[:, None])
    g2c = np.ascontiguousarray(g2[:, None])
    b2c = np.ascontiguousarray(beta2[:, None])

    # per-core residual slice (feature-major) and validity mask
    span = NTC * P
    in_maps = []
    for c in range(NCORES):
        c0 = c * span
        v = min(max(N - c0, 0), span)
        xct = np.zeros((D, span), np.float32)
        maskb = np.zeros((P, span), np.float32)
        if v > 0:
            xct[:, :v] = x.T[:, c0:c0 + v]
            maskb[:, :v] = 1.0
        in_maps.append({
            "xt": xt, "xbt": xbt,
            "wxt": wxt, "wbt": wbt, "w1t": w1t, "w2t": w2t,
            "linbb": linbb, "iotab": iotab, "ident": ident,
            "g1c": g1c, "b1c": b1c, "g2c": g2c, "b2c": b2c,
            "xct": xct, "maskb": maskb,
            "briw": _pack_idx(bri_pad[c].astype(np.int16)),
            "dstloc": np.ascontiguousarray(dloc_T[c]),
            "seld": sels[c],
            "woffs": woffs[c],
        })
    meta = (tuple(int(b) for b in NB_t), 1.0 + eps)
    return in_maps, meta


def _batch_meta(NB_t):
    """Flat batch list: (lt, first_of_tile, last_of_tile)."""
    out = []
    for lt in range(NTC):
        for k in range(NB_t[lt]):
            out.append((lt, k == 0, k == NB_t[lt] - 1))
    return out


def _build(meta):
    import concourse.bacc as bacc
    import concourse.bass as bass
    import concourse.mybir as mybir
    import concourse.tile as tile

    NB_t, resid_scale = meta
    NBAT = sum(NB_t)
    S = NBAT * P
    f32 = mybir.dt.float32
    bf16 = mybir.dt.bfloat16
    fp8 = mybir.dt.float8e4
    i16 = mybir.dt.int16
    i32 = mybir.dt.int32
    Alu = mybir.AluOpType
    Act = mybir.ActivationFunctionType
    span = NTC * P
    bmeta = _batch_meta(NB_t)
    nchunks = (NBAT + CHUNK - 1) // CHUNK
    assert NBAT % CHUNK == 0

    scratch = int(os.environ.get("KSCRATCH", "16384"))
    nc = bacc.Bacc("TRN2", target_bir_lowering=False, debug=False,
                   num_devices=NCORES, num_swdge_queues=NQ,
                   dynamic_dma_scratch_size=scratch)
    nc.sbuf_top = min(nc.sbuf_top, 192 * 1024)

    def din(name, shape, dt=f32):
        return nc.dram_tensor(name, shape, dt, kind="ExternalInput")

    xt_d = din("xt", [D, N2], bf16)
    xbt_d = din("xbt", [D, NB2], bf16)
    wxt_d = din("wxt", [D, D], bf16)
    wbt_d = din("wbt", [D, D], bf16)
    w1t_d = din("w1t", [D, D])
    linbb_d = din("linbb", [P, D])
    iotab_d = din("iotab", [P, CHUNK * P], bf16)
    xct_d = din("xct", [D, span])
    briw_d = din("briw", [128, S // 16], i16)
    dstloc_d = din("dstloc", [P, NBAT], bf16)
    seld_d = din("seld", [P, NBAT * 2 * P], fp8)
    woffs_d = din("woffs", [1, 2 * NBAT], i32)
    h_out_d = nc.dram_tensor("h_out", [P, span], f32, kind="ExternalOutput")
    stat_out_d = nc.dram_tensor("stat_out", [P, 2], f32, kind="ExternalOutput")

    eW_d = nc.dram_tensor("eW_tab", [NB2, D], bf16)

    with tile.TileContext(nc) as tc:
        with (
            tc.tile_pool(name="consts", bufs=1) as cp,
            tc.tile_pool(name="pa_src", bufs=2) as pa_src,
            tc.tile_pool(name="pa_stg", bufs=2) as pa_stg,
            tc.tile_pool(name="psA", bufs=2, space="PSUM") as psA,
            tc.tile_pool(name="psX", bufs=3, space="PSUM") as psX,
            tc.tile_pool(name="psB", bufs=2, space="PSUM") as psB,
            tc.tile_pool(name="gx", bufs=GBUFS) as gxp,
            tc.tile_pool(name="sel", bufs=3) as selp,
            tc.tile_pool(name="oh", bufs=3) as ohp,
            tc.tile_pool(name="msg", bufs=6) as msgp,
            tc.tile_pool(name="full", bufs=1) as fullp,
            tc.tile_pool(name="small", bufs=1) as smallp,
        ):
            def load_const(name, dram, shape, dt=f32, eng=None):
                t = cp.tile(shape, dt, tag=f"c_{name}")
                (eng or nc.sync).dma_start(t[:], dram[:])
                return t

            wxt = load_const("wxt", wxt_d, [D, D], bf16)
            wbt = load_const("wbt", wbt_d, [D, D], bf16)
            w1t = load_const("w1t", w1t_d, [D, D], eng=nc.scalar)
            linbb = load_const("linbb", linbb_d, [P, D])
            iotab = load_const("iotab", iotab_d, [P, CHUNK * P], bf16)
            xct = load_const("xct", xct_d, [D, span], eng=nc.scalar)
            briw = load_const("briw", briw_d, [128, S // 16], i16,
                              eng=nc.scalar)
            dstloc = load_const("dstloc", dstloc_d, [P, NBAT], bf16,
                                eng=nc.scalar)
            woffs = load_const("woffs", woffs_d, [1, 2 * NBAT], i32,
                               eng=nc.scalar)
            xw = cp.tile([P, NWX * P], bf16, tag="xw_tab")

            # ---------------- Phase A: eW table in DRAM, xW in SBUF --------
            CW = 2048

            # eW = x_bridge @ Wb.T + lin_b  ->  DRAM rows (bf16)
            for c0 in range(0, NB2, CW):
                w = min(CW, NB2 - c0)
                s = pa_src.tile([D, CW], bf16, tag="pa_src")
                nc.sync.dma_start(s[:, :w], xbt_d[:, c0:c0 + w])
                g = pa_stg.tile([P, CW], bf16, tag="pa_stg")
                for q0 in range(0, w, 512):
                    qw = min(512, w - q0)
                    ps = psA.tile([P, 512], f32, tag="psA")
                    for j in range(qw // P):
                        nc.tensor.matmul(
                            ps[:, j * P:(j + 1) * P],
                            s[:, q0 + j * P:q0 + (j + 1) * P], wbt[:])
                    lb = linbb[:].rearrange(
                        "p (b d) -> p b d", b=1).to_broadcast((P, qw // P, P))
                    nc.vector.tensor_tensor(
                        g[:, q0:q0 + qw].rearrange("p (b c) -> p b c", c=P),
                        ps[:, :qw].rearrange("p (b c) -> p b c", c=P),
                        lb, Alu.add)
                nc.scalar.dma_start(
                    eW_d[c0:c0 + w, :].rearrange("(g p) d -> p g d", p=P),
                    g[:, :w].rearrange("p (g d) -> p g d", d=P))

            # xW = x @ Wx.T -> SBUF [128, 80*128] (row-in-window, win*feat)
            for c0 in range(0, N2, CW):
                w = min(CW, N2 - c0)
                s = pa_src.tile([D, CW], bf16, tag="pa_src")
                nc.sync.dma_start(s[:, :w], xt_d[:, c0:c0 + w])
                for q0 in range(0, w, 512):
                    qw = min(512, w - q0)
                    ps = psA.tile([P, 512], f32, tag="psA")
                    for j in range(qw // P):
                        nc.tensor.matmul(
                            ps[:, j * P:(j + 1) * P],
                            s[:, q0 + j * P:q0 + (j + 1) * P], wxt[:])
                    nc.scalar.activation(
                        xw[:, c0 + q0:c0 + q0 + qw], ps[:, :qw], Act.Copy)

            # rhs AP template for the reg-offset expand matmuls
            xw0 = xw[:, 0:P]
            assert not xw0.offset, f"xw window-0 offset {xw0.offset} != 0"
            xw_ap = [list(dim) for dim in xw0.ap]

            # ---------------- Phase B: expand + gather + scatter ----------
            outT = fullp.tile([P, span], f32, tag="outT")
            h1 = fullp.tile([P, span], f32, tag="h")
            pstat = smallp.tile([P, NTC], f32, tag="pstat")
            pstat2 = smallp.tile([P, NTC], f32, tag="pstat2")

            wrs = [nc.tensor.allocate_register() for _ in range(4)]
            aggT = None

            for ci in range(nchunks):
                B0 = ci * CHUNK
                cb = min(CHUNK, NBAT - B0)
                gx = gxp.tile([P, CHUNK, D], bf16, tag="gx")
                nc.gpsimd.dma_gather(
                    gx[:, :cb, :], eW_d[:],
                    briw[:, B0 * 8:(B0 + cb) * 8],
                    cb * P, cb * P, D, single_packet=False,
                    queue_num=ci % NQ)
                sel = selp.tile([P, CHUNK, 2, P], fp8, tag="sel")
                nc.sync.dma_start(
                    sel[:, :cb, :, :].rearrange("p c k d -> p (c k d)"),
                    seld_d[:, B0 * 2 * P:(B0 + cb) * 2 * P])
                oh = ohp.tile([P, CHUNK * P], bf16, tag="oh")
                dl = dstloc[:, B0:B0 + cb]
                nc.vector.tensor_tensor(
                    oh[:, :cb * P].rearrange("p (b c) -> p b c", c=P),
                    iotab[:, :cb * P].rearrange("p (b c) -> p b c", c=P),
                    dl.to_broadcast((P, cb, P)), Alu.is_equal)

                for g0 in range(0, cb, 2):
                    gn = min(2, cb - g0)
                    psx = psX.tile([P, 2 * P], f32, tag="psx")
                    for j in range(gn):
                        b = g0 + j
                        Bg = B0 + b
                        for k in range(2):
                            wr = wrs[(2 * b + k) % 4]
                            nc.tensor.reg_load(
                                wr, woffs[0:1, 2 * Bg + k:2 * Bg + k + 1])
                            rhs = bass.AP(xw0.tensor, wr, xw_ap)
                            nc.tensor.matmul(
                                psx[:, j * P:(j + 1) * P],
                                sel[:, b, k, :], rhs,
                                start=(k == 0), stop=(k == 1))
                    msg = msgp.tile([P, 2, D], bf16, tag="msg")
                    nc.vector.tensor_tensor(
                        msg[:, :gn, :],
                        psx[:, :gn * P].rearrange("p (g d) -> p g d", d=P),
                        gx[:, g0:g0 + gn, :], Alu.add)
                    nc.scalar.activation(msg[:, :gn, :], msg[:, :gn, :],
                                         Act.Relu)
                    for j in range(gn):
                        b = g0 + j
                        lt, first, last = bmeta[B0 + b]
                        if first:
                            aggT = psB.tile([P, P], f32, tag="aggT")
                        nc.tensor.matmul(
                            aggT[:], msg[:, j, :], oh[:, b * P:(b + 1) * P],
                            start=first, stop=last)
                        if last:
                            sl = slice(lt * P, (lt + 1) * P)
                            nc.vector.scalar_tensor_tensor(
                                outT[:, sl], xct[:, sl], float(resid_scale),
                                aggT[:], Alu.mult, Alu.add)
                            ps = psA.tile([P, 512], f32, tag="psA")
                            nc.tensor.matmul(ps[:, :P], w1t[:], outT[:, sl])
                            nc.scalar.activation(h1[:, sl], ps[:, :P],
                                                 Act.Copy)
                            sqt = ohp.tile([P, P], f32, tag="sqt")
                            nc.vector.tensor_tensor(sqt[:], h1[:, sl],
                                                    h1[:, sl], Alu.mult)
                            nc.vector.reduce_sum(pstat[:, lt:lt + 1],
                                                 h1[:, sl],
                                                 axis=mybir.AxisListType.X)
                            nc.vector.reduce_sum(pstat2[:, lt:lt + 1], sqt[:],
                                                 axis=mybir.AxisListType.X)
                            nc.sync.dma_start(h_out_d[:, sl], h1[:, sl])

            # fold the per-tile partials; 8-way BN stat reduction on host
            s_stat = smallp.tile([P, 2], f32, tag="stat1")
            nc.vector.reduce_sum(s_stat[:, 0:1], pstat[:],
                                 axis=mybir.AxisListType.X)
            nc.vector.reduce_sum(s_stat[:, 1:2], pstat2[:],
                                 axis=mybir.AxisListType.X)
            nc.sync.dma_start(stat_out_d[:], s_stat[:])

    nc.compile()
    return nc


def _bn_scale_shift(nc, mybir, smallp, red, gcol, bcol, idx):
    """Device-side BN coefficients from reduced stats: a = g*rstd, sh = b - mu*a."""
    f32 = mybir.dt.float32
    Alu = mybir.AluOpType
    Act = mybir.ActivationFunctionType
    mu = smallp.tile([P, 1], f32, tag=f"mu{idx}")
    nc.vector.tensor_scalar(mu[:], red[:, 0:1], 1.0 / N, None, Alu.mult)
    mu2 = smallp.tile([P, 1], f32, tag=f"mu2{idx}")
    nc.vector.tensor_tensor(mu2[:], mu[:], mu[:], Alu.mult)
    e2 = smallp.tile([P, 1], f32, tag=f"e2{idx}")
    nc.vector.tensor_scalar(e2[:], red[:, 1:2], 1.0 / N, None, Alu.mult)
    var = smallp.tile([P, 1], f32, tag=f"var{idx}")
    nc.vector.tensor_tensor(var[:], e2[:], mu2[:], Alu.subtract)
    vep = smallp.tile([P, 1], f32, tag=f"vep{idx}")
    nc.vector.tensor_scalar(vep[:], var[:], BN_EPS, None, Alu.add)
    std = smallp.tile([P, 1], f32, tag=f"std{idx}")
    nc.scalar.activation(std[:], vep[:], Act.Sqrt)
    rstd = smallp.tile([P, 1], f32, tag=f"rstd{idx}")
    nc.vector.reciprocal(rstd[:], std[:])
    a = smallp.tile([P, 1], f32, tag=f"a{idx}")
    nc.vector.tensor_tensor(a[:], gcol[:], rstd[:], Alu.mult)
    tmp = smallp.tile([P, 1], f32, tag=f"tmp{idx}")
    nc.vector.tensor_tensor(tmp[:], mu[:], a[:], Alu.mult)
    sh = smallp.tile([P, 1], f32, tag=f"sh{idx}")
    nc.vector.tensor_tensor(sh[:], bcol[:], tmp[:], Alu.subtract)
    return a, sh


def _build_phase2():
    """NEFF2: h1n = mask*relu(BN1(h1)); h2 = h1n @ w2.T; per-core stats of h2."""
    import concourse.bacc as bacc
    import concourse.mybir as mybir
    import concourse.tile as tile

    f32 = mybir.dt.float32
    Alu = mybir.AluOpType
    Act = mybir.ActivationFunctionType
    span = NTC * P

    nc = bacc.Bacc("TRN2", target_bir_lowering=False, debug=False,
                   num_devices=NCORES)
    nc.sbuf_top = min(nc.sbuf_top, 192 * 1024)

    def din(name, shape):
        return nc.dram_tensor(name, shape, f32, kind="ExternalInput")

    h_d = din("h_in", [P, span])
    red_d = din("red", [P, 2])
    w2t_d = din("w2t", [D, D])
    g1c_d = din("g1c", [P, 1]); b1c_d = din("b1c", [P, 1])
    maskb_d = din("maskb", [P, span])
    h_out_d = nc.dram_tensor("h_out", [P, span], f32, kind="ExternalOutput")
    stat_out_d = nc.dram_tensor("stat_out", [P, 2], f32, kind="ExternalOutput")

    with tile.TileContext(nc) as tc:
        with (
            tc.tile_pool(name="consts", bufs=1) as cp,
            tc.tile_pool(name="psA", bufs=2, space="PSUM") as psA,
            tc.tile_pool(name="full", bufs=1) as fullp,
            tc.tile_pool(name="small", bufs=1) as smallp,
        ):
            def load_const(name, dram, shape):
                t = cp.tile(shape, f32, tag=f"c_{name}")
                nc.sync.dma_start(t[:], dram[:])
                return t

            h1 = load_const("h", h_d, [P, span])
            red = load_const("red", red_d, [P, 2])
            w2t = load_const("w2t", w2t_d, [D, D])
            g1c = load_const("g1c", g1c_d, [P, 1])
            b1c = load_const("b1c", b1c_d, [P, 1])
            maskb = load_const("maskb", maskb_d, [P, span])

            a, sh = _bn_scale_shift(nc, mybir, smallp, red, g1c, b1c, 0)
            hn = fullp.tile([P, span], f32, tag="hn")
            nc.scalar.activation(hn[:], h1[:], Act.Relu, bias=sh[:], scale=a[:])
            nc.vector.tensor_tensor(hn[:], hn[:], maskb[:], Alu.mult)

            h2 = fullp.tile([P, span], f32, tag="h2")
            for lt in range(NTC):
                sl = slice(lt * P, (lt + 1) * P)
                ps = psA.tile([P, P], f32, tag="psA")
                nc.tensor.matmul(ps[:], w2t[:], hn[:, sl])
                nc.scalar.activation(h2[:, sl], ps[:], Act.Copy)
            s_stat = smallp.tile([P, 2], f32, tag="stat2")
            nc.vector.reduce_sum(s_stat[:, 0:1], h2[:],
                                 axis=mybir.AxisListType.X)
            sq = fullp.tile([P, span], f32, tag="sq")
            nc.vector.tensor_tensor(sq[:], h2[:], h2[:], Alu.mult)
            nc.vector.reduce_sum(s_stat[:, 1:2], sq[:],
                                 axis=mybir.AxisListType.X)
            nc.sync.dma_start(stat_out_d[:], s_stat[:])
            nc.sync.dma_start(h_out_d[:], h2[:])

    nc.compile()
    return nc


def _build_phase3():
    """NEFF3: out = transpose(relu(BN2(h2)))."""
    import concourse.bacc as bacc
    import concourse.mybir as mybir
    import concourse.tile as tile

    f32 = mybir.dt.float32
    Act = mybir.ActivationFunctionType
    span = NTC * P

    nc = bacc.Bacc("TRN2", target_bir_lowering=False, debug=False,
                   num_devices=NCORES)
    nc.sbuf_top = min(nc.sbuf_top, 192 * 1024)

    def din(name, shape):
        return nc.dram_tensor(name, shape, f32, kind="ExternalInput")

    h_d = din("h_in", [P, span])
    red_d = din("red", [P, 2])
    g2c_d = din("g2c", [P, 1]); b2c_d = din("b2c", [P, 1])
    ident_d = din("ident", [P, P])
    out_d = nc.dram_tensor("out", [span, D], f32, kind="ExternalOutput")

    with tile.TileContext(nc) as tc:
        with (
            tc.tile_pool(name="consts", bufs=1) as cp,
            tc.tile_pool(name="psA", bufs=2, space="PSUM") as psA,
            tc.tile_pool(name="full", bufs=1) as fullp,
            tc.tile_pool(name="small", bufs=1) as smallp,
        ):
            def load_const(name, dram, shape):
                t = cp.tile(shape, f32, tag=f"c_{name}")
                nc.sync.dma_start(t[:], dram[:])
                return t

            h2 = load_const("h", h_d, [P, span])
            red = load_const("red", red_d, [P, 2])
            g2c = load_const("g2c", g2c_d, [P, 1])
            b2c = load_const("b2c", b2c_d, [P, 1])
            ident = load_const("ident", ident_d, [P, P])

            a, sh = _bn_scale_shift(nc, mybir, smallp, red, g2c, b2c, 1)
            hn = fullp.tile([P, span], f32, tag="hn")
            nc.scalar.activation(hn[:], h2[:], Act.Relu, bias=sh[:], scale=a[:])

            stg = fullp.tile([P, span], f32, tag="stg")
            for lt in range(NTC):
                sl = slice(lt * P, (lt + 1) * P)
                ps = psA.tile([P, P], f32, tag="psA")
                nc.tensor.transpose(ps[:], hn[:, sl], ident[:])
                nc.scalar.activation(stg[:, sl], ps[:], Act.Copy)
            nc.sync.dma_start(
                out_d[:].rearrange("(g p) d -> p g d", p=P),
                stg[:].rearrange("p (g d) -> p g d", d=P))

    nc.compile()
    return nc


def kernel(**inputs):
    global last_results
    from concourse.bass_utils import run_bass_kernel_spmd

    in_maps, meta = _host_prep(inputs)
    if meta not in _cache:
        _cache[meta] = _build(meta)
    if "p2" not in _cache2:
        _cache2["p2"] = _build_phase2()
        _cache2["p3"] = _build_phase3()
    nc1, nc2, nc3 = _cache[meta], _cache2["p2"], _cache2["p3"]
    cores = list(range(NCORES))
    trace = bool(os.environ.get("KERNEL_TRACE"))

    k1 = ("xt", "xbt", "wxt", "wbt", "w1t", "linbb", "iotab", "xct",
          "briw", "dstloc", "seld", "woffs")
    in1 = [{k: in_maps[c][k] for k in k1} for c in range(NCORES)]
    res1 = run_bass_kernel_spmd(nc1, in1, cores, trace=trace)
    red1 = np.sum([res1.results[c]["stat_out"] for c in range(NCORES)], axis=0)
    in2 = [{"h_in": res1.results[c]["h_out"], "red": red1,
            "w2t": in_maps[c]["w2t"], "g1c": in_maps[c]["g1c"],
            "b1c": in_maps[c]["b1c"], "maskb": in_maps[c]["maskb"]}
           for c in range(NCORES)]
    res2 = run_bass_kernel_spmd(nc2, in2, cores, trace=trace)
    red2 = np.sum([res2.results[c]["stat_out"] for c in range(NCORES)], axis=0)
    in3 = [{"h_in": res2.results[c]["h_out"], "red": red2,
            "g2c": in_maps[c]["g2c"], "b2c": in_maps[c]["b2c"],
            "ident": in_maps[c]["ident"]} for c in range(NCORES)]
    res3 = run_bass_kernel_spmd(nc3, in3, cores, trace=trace)

    last_results = (res1, res2, res3)
    out = np.concatenate([res3.results[c]["out"] for c in range(NCORES)], axis=0)
    return np.ascontiguousarray(out[:N])
